# revision 1
# baseline (speedup 1.0000x reference)
"""Trainium2 Bass kernel for AViT block (T=16,B=2,H=32,W=32,C=512, 8 heads).

Sharding: data-parallel over H (32 -> 4 rows per core, 8 cores).
Per-core token order: (b, hs, w, t) with t innermost -> attention groups
(head, b, hs, w-octet) are 128 consecutive tokens; instance-norm samples
(b, t) are stride-16 slices of the free dim. Branch computed in bf16
(output is x + 1e-6*branch, so branch precision is uncritical);
stats/softmax accumulation in fp32. Instance-norm statistics are
all-reduced across cores (2 x 128KB AllReduce).
"""

import math
import numpy as np

import concourse.bass as bass
import concourse.bacc as bacc
import concourse.tile as tile
from concourse import mybir
from concourse.bass_utils import run_bass_kernel_spmd

T, B, H, W, C = 16, 2, 32, 32, 512
NH, HD = 8, 64
NCORES = 8
HS = H // NCORES          # 4 H-rows per core
NTOK = T * B * HS * W     # 4096 tokens per core
SPA = HS * W              # 128 local spatial positions per sample
NT_TILES = NTOK // 128    # 32 token tiles; tile = (b, hs, woct)
NCC = C // 128            # 4 channel chunks
EPS = 1e-5
NEG = -30.0

f32 = mybir.dt.float32
bf16 = mybir.dt.bfloat16
AL = mybir.AluOpType

_CACHE = {}


def _bcast(t, offset, n):
    return bass.AP(tensor=t, offset=offset, ap=[[0, 128], [1, n]])


def build_program():
    nc = bacc.Bacc("TRN2", target_bir_lowering=False, debug=False,
                   num_devices=NCORES)
    dt = nc.dram_tensor
    x_d = dt("x", [T, B, HS, W, C], f32, kind="ExternalInput")
    wtin_d = dt("wtin", [NCC, 128, 3 * C], bf16, kind="ExternalInput")
    bin_d = dt("binp", [3 * C], f32, kind="ExternalInput")
    wtout_d = dt("wtout", [NCC, 128, C], bf16, kind="ExternalInput")
    beff_d = dt("beff", [1, C], bf16, kind="ExternalInput")
    qwb_d = dt("qwb", [4, C], f32, kind="ExternalInput")   # qw,qb,kw,kb rows
    n12_d = dt("n12", [4, C], f32, kind="ExternalInput")   # n1w,n1b,n2w,n2b
    bias_d = dt("biastab", [NH, 128, 128], bf16, kind="ExternalInput")
    y_d = dt("y", [T, B, HS, W, C], f32, kind="ExternalOutput")

    cc1_in = dt("cc1_in", [NCC, 128, 64], f32)
    cc1_out = dt("cc1_out", [NCC, 128, 64], f32, addr_space="Shared")
    cc2_in = dt("cc2_in", [NCC, 128, 64], f32)
    cc2_out = dt("cc2_out", [NCC, 128, 64], f32, addr_space="Shared")
    RG = [list(range(NCORES))]

    xr = x_d.ap().rearrange("t b h (wo w) c -> b h wo w t c", wo=W // 8)
    yr = y_d.ap().rearrange("t b h (wo w) c -> b h wo w t c", wo=W // 8)

    from contextlib import ExitStack
    with tile.TileContext(nc) as tc, ExitStack() as ctx:
        res = ctx.enter_context(tc.tile_pool(name="res", bufs=1))
        tp = ctx.enter_context(tc.tile_pool(name="tmp", bufs=3))
        sq2 = ctx.enter_context(tc.tile_pool(name="sq2", bufs=1))
        tp2 = ctx.enter_context(tc.tile_pool(name="tmp2", bufs=2))
        sp = ctx.enter_context(tc.tile_pool(name="small", bufs=4))
        pa = ctx.enter_context(tc.tile_pool(name="pa", bufs=6, space="PSUM"))
        pb = ctx.enter_context(tc.tile_pool(name="pb", bufs=2, space="PSUM"))

        # ---- static tiles ----
        wt_in = res.tile([128, NCC, 3 * C], bf16, tag="wt_in")
        wt_out = res.tile([128, NCC, C], bf16, tag="wt_out")
        for cci in range(NCC):
            nc.sync.dma_start(out=wt_in[:, cci, :], in_=wtin_d[cci])
            nc.sync.dma_start(out=wt_out[:, cci, :], in_=wtout_d[cci])
        beff_t = res.tile([1, C], bf16, tag="beff")
        nc.sync.dma_start(out=beff_t[:], in_=beff_d[:, :])
        b_in_t = res.tile([128, 3, C], f32, tag="b_in")
        for ob in range(3):
            nc.sync.dma_start(out=b_in_t[:, ob, :], in_=_bcast(bin_d, ob * C, C))
        qwb_f = sq2.tile([128, C], f32, tag="qwbf")
        qwb_t = res.tile([128, 4, C], bf16, tag="qwb")
        for i in range(4):
            nc.sync.dma_start(out=qwb_f[:], in_=_bcast(qwb_d, i * C, C))
            nc.vector.tensor_copy(out=qwb_t[:, i, :], in_=qwb_f[:])
        n12_t = res.tile([128, 4, NCC], f32, tag="n12")
        for i in range(4):
            for cci in range(NCC):
                nc.sync.dma_start(
                    out=n12_t[:, i, cci:cci + 1],
                    in_=bass.AP(tensor=n12_d, offset=i * C + cci * 128,
                                ap=[[1, 128], [1, 1]]))
        bias_t = res.tile([128, NH, 128], bf16, tag="bias")
        for h in range(NH):
            nc.sync.dma_start(out=bias_t[:, h, :], in_=bias_d[h])
        ones_c = res.tile([128, 1], bf16, tag="ones")
        nc.vector.memset(ones_c[:], 1.0)
        ones_r = res.tile([1, 128], bf16, tag="onesr")
        nc.vector.memset(ones_r[:], 1.0)
        eps_t = res.tile([128, 1], f32, tag="eps")
        nc.vector.memset(eps_t[:], EPS)

        # ---- resident activations (xT doubles as aT later) ----
        xT = res.tile([128, NCC, NTOK], bf16, tag="xT")
        v_tm = res.tile([128, NT_TILES, C], bf16, tag="v_tm")
        qnT = res.tile([128, NCC, NTOK], bf16, tag="qnT")
        knT = res.tile([128, NCC, NTOK], bf16, tag="knT")
        aT = xT  # alias: xn is fully consumed by QKV before aT is written

        def tview(ap128):  # [128, NTOK] -> [128, b, t, spatial]
            return ap128.rearrange("p (b s t) -> p b t s", b=B, s=SPA, t=T)

        # ================= load x, cast, transpose =================
        for tt in range(NT_TILES):
            b, hs, wo = tt // 16, (tt // 4) % 4, tt % 4
            xf = tp.tile([128, C], f32, tag="xf")
            nc.sync.dma_start(out=xf[:], in_=xr[b, hs, wo])
            xq = tp.tile([128, C], bf16, tag="xq")
            nc.vector.tensor_copy(out=xq[:], in_=xf[:])
            for cci in range(NCC):
                nc.sync.dma_start_transpose(
                    out=xT[:, cci, tt * 128:(tt + 1) * 128],
                    in_=xq[:, cci * 128:(cci + 1) * 128])

        # ================= instance-norm stats + AllReduce =================
        def stats_allreduce(src, ccin, ccout, tg):
            for cci in range(NCC):
                red = tp2.tile([128, 64], f32, tag="red" + tg)
                nc.vector.tensor_reduce(out=red[:, 0:32],
                                        in_=tview(src[:, cci, :]),
                                        axis=mybir.AxisListType.X, op=AL.add)
                sq = sq2.tile([128, NTOK], bf16, tag="sq")
                nc.vector.tensor_mul(sq[:], src[:, cci, :], src[:, cci, :])
                nc.vector.tensor_reduce(out=red[:, 32:64], in_=tview(sq[:]),
                                        axis=mybir.AxisListType.X, op=AL.add)
                nc.sync.dma_start(out=ccin[cci], in_=red[:])
            nc.gpsimd.collective_compute(
                "AllReduce", AL.add, replica_groups=RG,
                ins=[ccin[:, :, :]], outs=[ccout[:, :, :]])
            scl, sft = [], []
            for cci in range(NCC):
                g = tp2.tile([128, 64], f32, tag="g" + tg)
                nc.sync.dma_start(out=g[:], in_=ccout[cci])
                mean = sp.tile([128, 32], f32, tag="mean" + tg)
                nc.scalar.mul(out=mean[:], in_=g[:, 0:32],
                              mul=1.0 / (SPA * NCORES))
                msq = tp2.tile([128, 32], f32, tag="msq" + tg)
                nc.vector.tensor_mul(msq[:], mean[:], mean[:])
                var = tp2.tile([128, 32], f32, tag="var" + tg)
                nc.vector.scalar_tensor_tensor(
                    out=var[:], in0=g[:, 32:64], scalar=1.0 / (SPA * NCORES),
                    in1=msq[:], op0=AL.mult, op1=AL.subtract)
                nc.scalar.activation(out=var[:], in_=var[:],
                                     func=mybir.ActivationFunctionType.Sqrt,
                                     bias=eps_t[:], scale=1.0)
                rstd = sp.tile([128, 32], f32, tag="rstd" + tg)
                nc.vector.reciprocal(out=rstd[:], in_=var[:])
                scl.append(rstd)
                sft.append(mean)
            return scl, sft

        def norm_apply(src, scl, sft, w_i, b_i):
            for cci in range(NCC):
                s1 = sp.tile([128, 32], f32, tag="s1")
                nc.vector.tensor_scalar(
                    out=s1[:], in0=scl[cci][:],
                    scalar1=n12_t[:, w_i, cci:cci + 1], scalar2=None,
                    op0=AL.mult)
                t1 = sp.tile([128, 32], f32, tag="t1")
                nc.vector.tensor_mul(t1[:], sft[cci][:], s1[:])
                nc.vector.tensor_scalar(
                    out=t1[:], in0=t1[:], scalar1=-1.0,
                    scalar2=n12_t[:, b_i, cci:cci + 1],
                    op0=AL.mult, op1=AL.add)
                v = tview(src[:, cci, :])
                for b in range(B):
                    for t in range(T):
                        s = b * T + t
                        nc.vector.tensor_scalar(
                            out=v[:, b, t, :], in0=v[:, b, t, :],
                            scalar1=s1[:, s:s + 1], scalar2=t1[:, s:s + 1],
                            op0=AL.mult, op1=AL.add)

        scl, sft = stats_allreduce(xT, cc1_in, cc1_out, "a")
        norm_apply(xT, scl, sft, 0, 1)

        # ========== QKV projection + q/k layernorm + transposes ==========
        def layernorm_qk(qs, wrow, brow):
            h8 = qs.rearrange("p (h d) -> p h d", h=NH)
            su = sp.tile([128, NH], f32, tag="lnsum")
            nc.vector.tensor_reduce(out=su[:], in_=h8,
                                    axis=mybir.AxisListType.X, op=AL.add)
            mu = sp.tile([128, NH], f32, tag="lnmu")
            nc.scalar.mul(out=mu[:], in_=su[:], mul=1.0 / HD)
            sq = tp2.tile([128, C], bf16, tag="lnsq")
            nc.vector.tensor_mul(sq[:], qs, qs)
            ss = sp.tile([128, NH], f32, tag="lnss")
            nc.vector.tensor_reduce(
                out=ss[:], in_=sq.rearrange("p (h d) -> p h d", h=NH),
                axis=mybir.AxisListType.X, op=AL.add)
            msq = sp.tile([128, NH], f32, tag="lnmsq")
            nc.vector.tensor_mul(msq[:], mu[:], mu[:])
            var = sp.tile([128, NH], f32, tag="lnvar")
            nc.vector.scalar_tensor_tensor(
                out=var[:], in0=ss[:], scalar=1.0 / HD, in1=msq[:],
                op0=AL.mult, op1=AL.subtract)
            nc.scalar.activation(out=var[:], in_=var[:],
                                 func=mybir.ActivationFunctionType.Sqrt,
                                 bias=eps_t[:], scale=1.0)
            rs = sp.tile([128, NH], f32, tag="lnrs")
            nc.vector.reciprocal(out=rs[:], in_=var[:])
            t1 = tp2.tile([128, C], bf16, tag="lnt1")
            for h in range(NH):
                nc.vector.tensor_scalar(
                    out=t1[:, h * HD:(h + 1) * HD],
                    in0=qs[:, h * HD:(h + 1) * HD],
                    scalar1=mu[:, h:h + 1], scalar2=rs[:, h:h + 1],
                    op0=AL.subtract, op1=AL.mult)
            nc.vector.tensor_mul(t1[:], t1[:], qwb_t[:, wrow, :])
            nc.vector.tensor_add(qs, t1[:], qwb_t[:, brow, :])

        for tt in range(NT_TILES):
            qt = tp2.tile([128, C], bf16, tag="qt")
            kt = tp2.tile([128, C], bf16, tag="kt")
            for ob, dst in ((0, qt[:]), (1, kt[:]), (2, v_tm[:, tt, :])):
                ps = pb.tile([128, C], f32, tag="big")
                for cci in range(NCC):
                    nc.tensor.matmul(
                        ps[:], xT[:, cci, tt * 128:(tt + 1) * 128],
                        wt_in[:, cci, ob * C:(ob + 1) * C],
                        start=(cci == 0), stop=(cci == NCC - 1))
                nc.vector.tensor_add(dst, ps[:], b_in_t[:, ob, :])
            layernorm_qk(qt[:], 0, 1)
            layernorm_qk(kt[:], 2, 3)
            for cci in range(NCC):
                nc.sync.dma_start_transpose(
                    out=qnT[:, cci, tt * 128:(tt + 1) * 128],
                    in_=qt[:, cci * 128:(cci + 1) * 128])
                nc.sync.dma_start_transpose(
                    out=knT[:, cci, tt * 128:(tt + 1) * 128],
                    in_=kt[:, cci * 128:(cci + 1) * 128])

        # ================= attention =================
        for tt in range(NT_TILES):
            t0 = tt * 128
            at = tp2.tile([128, C], bf16, tag="at")
            for h in range(NH):
                cci, po = h // 2, (h % 2) * HD
                kk = knT[po:po + HD, cci, t0:t0 + 128]
                qq = qnT[po:po + HD, cci, t0:t0 + 128]
                sps = pa.tile([128, 128], f32, tag="att")
                nc.tensor.matmul(sps[:], kk, qq, start=True, stop=True)
                sb = tp2.tile([128, 128], f32, tag="ssb")
                nc.vector.tensor_add(sb[:], sps[:], bias_t[:, h, :])
                ah = tp2.tile([128, 128], bf16, tag="ahat")
                nc.scalar.activation(out=ah[:], in_=sb[:],
                                     func=mybir.ActivationFunctionType.Exp)
                pd = pa.tile([128, 1], f32, tag="att")
                nc.tensor.matmul(pd[:], ah[:], ones_c[:], start=True, stop=True)
                po2 = pa.tile([128, HD], f32, tag="att")
                nc.tensor.matmul(po2[:], ah[:],
                                 v_tm[:, tt, h * HD:(h + 1) * HD],
                                 start=True, stop=True)
                rd = sp.tile([128, 1], f32, tag="rd")
                nc.vector.reciprocal(out=rd[:], in_=pd[:])
                nc.vector.tensor_scalar(
                    out=at[:, h * HD:(h + 1) * HD], in0=po2[:],
                    scalar1=rd[:], scalar2=None, op0=AL.mult)
            for cci in range(NCC):
                nc.sync.dma_start_transpose(
                    out=aT[:, cci, t0:t0 + 128],
                    in_=at[:, cci * 128:(cci + 1) * 128])

        # ========== norm2 + output proj + residual ==========
        scl2, sft2 = stats_allreduce(aT, cc2_in, cc2_out, "b")
        norm_apply(aT, scl2, sft2, 2, 3)

        for tt in range(NT_TILES):
            b, hs, wo = tt // 16, (tt // 4) % 4, tt % 4
            ps = pb.tile([128, C], f32, tag="big")
            for cci in range(NCC):
                nc.tensor.matmul(ps[:], aT[:, cci, tt * 128:(tt + 1) * 128],
                                 wt_out[:, cci, :],
                                 start=(cci == 0), stop=False)
            nc.tensor.matmul(ps[:], ones_r[:], beff_t[:], start=False,
                             stop=True)
            xf = tp2.tile([128, C], f32, tag="xf2")
            nc.sync.dma_start(out=xf[:], in_=xr[b, hs, wo])
            ysb = tp2.tile([128, C], f32, tag="ysb")
            nc.vector.tensor_add(ysb[:], ps[:], xf[:])
            nc.sync.dma_start(out=yr[b, hs, wo], in_=ysb[:])

    nc.compile()
    return nc


def _host_prep(inputs):
    w_in = np.asarray(inputs["w_in"], np.float32)
    b_in = np.asarray(inputs["b_in"], np.float32)
    w_out = np.asarray(inputs["w_out"], np.float32)
    b_out = np.asarray(inputs["b_out"], np.float32)
    gamma = np.asarray(inputs["gamma"], np.float32)
    rel_emb = np.asarray(inputs["rel_emb"], np.float32)

    perm = np.zeros(3 * C, np.int64)
    for he in range(NH):
        for d in range(HD):
            perm[he * HD + d] = he * 192 + d
            perm[C + he * HD + d] = he * 192 + 64 + d
            perm[2 * C + he * HD + d] = he * 192 + 128 + d
    w_eff = w_in[perm]
    b_eff_in = b_in[perm]
    wtin = np.ascontiguousarray(w_eff.T).reshape(NCC, 128, 3 * C)
    wtout = np.ascontiguousarray((w_out * gamma[:, None]).T).reshape(NCC, 128, C)
    beff = (b_out * gamma).reshape(1, C)

    sc = HD ** -0.5
    qwb = np.stack([
        np.tile(np.asarray(inputs["qnorm_w"], np.float32), NH) * sc,
        np.tile(np.asarray(inputs["qnorm_b"], np.float32), NH) * sc,
        np.tile(np.asarray(inputs["knorm_w"], np.float32), NH),
        np.tile(np.asarray(inputs["knorm_b"], np.float32), NH)])
    n12 = np.stack([np.asarray(inputs["norm1_w"], np.float32),
                    np.asarray(inputs["norm1_b"], np.float32),
                    np.asarray(inputs["norm2_w"], np.float32),
                    np.asarray(inputs["norm2_b"], np.float32)])

    pos = np.arange(T)
    rp = pos[None, :] - pos[:, None]
    n = -rp
    nb = 16
    ret = (n < 0).astype(np.int64) * nb
    n = np.abs(n)
    mx = nb // 2
    vl = mx + (np.log(np.maximum(n, 1).astype(np.float32) / mx)
               / math.log(32 / mx) * (nb - mx)).astype(np.int64)
    vl = np.minimum(vl, nb - 1)
    bucket = ret + np.where(n < mx, n, vl)
    bias = rel_emb[bucket]                            # [i, j, h]
    biastab = np.full((NH, 128, 128), NEG, np.float32)
    for h in range(NH):
        bt = bias[:, :, h].T
        for p in range(8):
            biastab[h, 16 * p:16 * p + 16, 16 * p:16 * p + 16] = bt
    return dict(wtin=wtin, binp=b_eff_in, wtout=wtout, beff=beff, qwb=qwb,
                n12=n12, biastab=biastab)


def _base_maps(inputs):
    import ml_dtypes
    bfd = ml_dtypes.bfloat16
    hp = _host_prep(inputs)
    return dict(
        wtin=np.ascontiguousarray(hp["wtin"]).astype(bfd),
        binp=hp["binp"].astype(np.float32),
        wtout=np.ascontiguousarray(hp["wtout"]).astype(bfd),
        beff=hp["beff"].astype(bfd),
        qwb=hp["qwb"].astype(np.float32),
        n12=hp["n12"].astype(np.float32),
        biastab=hp["biastab"].astype(bfd))


def make_in_maps(inputs):
    base = _base_maps(inputs)
    x = np.asarray(inputs["x"], np.float32)
    in_maps = []
    for k in range(NCORES):
        m = dict(base)
        m["x"] = np.ascontiguousarray(x[:, :, HS * k:HS * (k + 1), :, :])
        in_maps.append(m)
    return in_maps


def kernel(**inputs):
    if "nc" not in _CACHE:
        _CACHE["nc"] = build_program()
    nc = _CACHE["nc"]
    res = run_bass_kernel_spmd(nc, make_in_maps(inputs),
                               core_ids=list(range(NCORES)))
    out = np.empty((T, B, H, W, C), np.float32)
    for k in range(NCORES):
        out[:, :, HS * k:HS * (k + 1), :, :] = res.results[k]["y"]
    return out

